# revision 6
# baseline (speedup 1.0000x reference)
"""Trainium2 Bass kernel for nn_Attention_73375221285454.

Multi-head self-attention (B=4, N=2048, D=768, H=12, DH=64) with key-padding
mask, distributed over 8 NeuronCores.

Sharding: core c handles batch b = c//2 and query half qh = c%2 (1024 query
rows). Each core computes K/V for its full batch (duplicated across the pair)
and attention + output projection for its query half; the 8 outputs tile the
full (4, 2048, 768) result with no collectives.

Host marshalling per core: x[b] is transposed (xkT for keys — sorted so that
unmasked keys come first, making trailing all-masked key tiles skippable —
and xqT for the query half in natural order); the bool mask becomes float
additive/multiplicative mask tables. Attention is permutation-invariant over
keys, so sorting keys (with the mask sorted identically) is exact.

Device algorithm per core (all matmuls in float32r ~ tf32):
  V    = (xkT.T @ Wv) stored as vaug [128, 16, 12, 65] with a ones column
  K^T  = Wk.T @ xkT  -> kT [128, 6, njt*128]    (only active key tiles)
  Q^T  = Wq.T @ xqT  -> qT [128, 6, 1024]
  per head h, active key tile jt:
    S^T[j, i] = K_h^T.T @ Q_h^T                (PSUM [128, 1024])
    P^T       = exp(0.125*S^T + cmneg[j])      (ACT; cmneg=-30000 if masked)
    O^T      += vaug[jt, h].T @ P^T            (PSUM [65, 1024]; row 64 = s[i])
  attnT_h = O^T[0:64] * (1/s)                  (DMA partition-bcast + DVE)
  out     = (attnT.T @ Wo) * rm01[i] + (1 - rm01[i]) (x) uniform_row
  where uniform_row = (mean_all_keys V) @ Wo reproduces the reference's
  uniform softmax over ALL keys for fully-masked query rows.

No max-subtraction is needed: logits are ~N(0,1) (exp can't overflow), masked
keys get exp(logit - 30000) == 0 exactly, and fully-masked query rows are
replaced by uniform_row at the end.
"""

import sys

sys.path.insert(0, "/opt/trn_rl_repo")

import numpy as np

import concourse.bass as bass  # noqa: F401
import concourse.mybir as mybir
import concourse.tile as tile
from concourse import bacc
from concourse.bass_utils import run_bass_kernel_spmd

P = 128
B, N, D = 4, 2048, 768
H, DH = 12, 64
NQ = N // 2              # queries per core
DC = D // P              # 6 contraction chunks
NJT_FULL = N // P        # 16 key tiles
NIT = NQ // P            # 8 query tiles
SCALE = DH ** -0.5       # 0.125
MASK_NEG = -30000.0
SORT_KEYS = True         # sort keys so all-masked key tiles are skipped

f32 = mybir.dt.float32
f32r = mybir.dt.float32r

_BUILD_CACHE = {}


def build(njt_act: int) -> "bacc.Bacc":
    """Build the SPMD program. njt_act = number of key tiles containing any
    unmasked key; trailing all-masked tiles contribute exactly zero to both
    softmax numerator and denominator and are skipped. V/meanV still cover
    all 16 tiles (masked-query rows need the mean over ALL keys)."""
    if njt_act in _BUILD_CACHE:
        return _BUILD_CACHE[njt_act]

    nk = njt_act * P  # active key columns

    nc = bacc.Bacc()
    xkT_d = nc.declare_dram_parameter("xkT", [D, N], f32, isOutput=False)
    xqT_d = nc.declare_dram_parameter("xqT", [D, NQ], f32, isOutput=False)
    wq_d = nc.declare_dram_parameter("Wq", [D, D], f32, isOutput=False)
    wk_d = nc.declare_dram_parameter("Wk", [D, D], f32, isOutput=False)
    wv_d = nc.declare_dram_parameter("Wv", [D, D], f32, isOutput=False)
    wo_d = nc.declare_dram_parameter("Wo", [D, D], f32, isOutput=False)
    # cmnegT[p, t] = 0.0 if key (t*128+p) unmasked else -30000.0
    cmneg_d = nc.declare_dram_parameter("cmnegT", [P, NJT_FULL], f32, isOutput=False)
    # rm01T[p, t] = 1.0 if query (t*128+p) unmasked else 0.0
    rm01_d = nc.declare_dram_parameter("rm01T", [P, NIT], f32, isOutput=False)
    # rmneg_row[0, i] = 1.0 - rm01[i]
    rmneg_d = nc.declare_dram_parameter("rmneg_row", [1, NQ], f32, isOutput=False)
    out_d = nc.declare_dram_parameter("out", [NQ, D], f32, isOutput=True)

    with tile.TileContext(nc) as tc:
        with tc.tile_pool(name="persist", bufs=1) as persist:
            # small persistent tiles
            cmneg = persist.tile([P, NJT_FULL], f32)
            nc.sync.dma_start(out=cmneg, in_=cmneg_d.ap())
            rm01 = persist.tile([P, NIT], f32)
            nc.sync.dma_start(out=rm01, in_=rm01_d.ap())
            rmneg_row = persist.tile([1, NQ], f32r)
            nc.sync.dma_start(out=rmneg_row, in_=rmneg_d.ap().bitcast(f32r))
            ones_f = persist.tile([P, H], f32)
            nc.vector.memset(ones_f, 1.0)
            ones_r = persist.tile([P, 1], f32r)
            nc.vector.tensor_copy(ones_r, ones_f[:, 0:1])
            id1 = persist.tile([1, 1], f32)
            nc.vector.memset(id1, 1.0)

            qT = persist.tile([P, DC, NQ], f32r)
            vaug = persist.tile([P, NJT_FULL, H, DH + 1], f32r)
            kT = persist.tile([P, DC, nk], f32r)
            attnT = persist.tile([P, DC, NQ], f32r)
            mvT_sb = persist.tile([P, DC], f32r)   # meanV^T (already / N)
            mv_row = persist.tile([1, D], f32)

            # ---------------- phase 1: Q projection ----------------
            with tc.tile_pool(name="xq_pool", bufs=1) as xq_pool, \
                 tc.tile_pool(name="wst1", bufs=2) as wst1, \
                 tc.tile_pool(name="psp1", bufs=3, space="PSUM") as psp1:
                xqT = xq_pool.tile([P, DC, NQ], f32r)
                nc.sync.dma_start(
                    out=xqT, in_=xqT_d.rearrange("(c p) n -> p c n", p=P).bitcast(f32r)
                )
                for hdt in range(DC):
                    wq_t = wst1.tile([P, DC, P], f32r, tag="wstream")
                    nc.sync.dma_start(
                        out=wq_t,
                        in_=wq_d.rearrange("(c p) e -> p c e", p=P)[
                            :, :, hdt * P : (hdt + 1) * P
                        ].bitcast(f32r),
                    )
                    for nch in range(NQ // 512):
                        ps = psp1.tile([P, 512], f32, tag="psproj")
                        for dc in range(DC):
                            nc.tensor.matmul(
                                ps,
                                wq_t[:, dc, :],
                                xqT[:, dc, nch * 512 : (nch + 1) * 512],
                                start=(dc == 0),
                                stop=(dc == DC - 1),
                            )
                        nc.vector.tensor_copy(
                            qT[:, hdt, nch * 512 : (nch + 1) * 512], ps
                        )

            # ---------------- phases 2+3: V and K projections ----------------
            with tc.tile_pool(name="xk_pool", bufs=1) as xk_pool:
                xkT = xk_pool.tile([P, DC, N], f32r)
                nc.sync.dma_start(
                    out=xkT, in_=xkT_d.rearrange("(c p) n -> p c n", p=P).bitcast(f32r)
                )
                # V projection + meanV
                with tc.tile_pool(name="wv_pool", bufs=1) as wv_pool, \
                     tc.tile_pool(name="psp2", bufs=2, space="PSUM") as psp2, \
                     tc.tile_pool(name="psmv", bufs=1, space="PSUM") as psmv:
                    wv_sb = wv_pool.tile([P, DC, D], f32r)
                    nc.sync.dma_start(
                        out=wv_sb,
                        in_=wv_d.rearrange("(c p) e -> p c e", p=P).bitcast(f32r),
                    )
                    for jt in range(NJT_FULL):
                        psv = psp2.tile([P, D], f32, tag="psv")
                        for dc in range(DC):
                            nc.tensor.matmul(
                                psv[:, 0:512],
                                xkT[:, dc, jt * P : (jt + 1) * P],
                                wv_sb[:, dc, 0:512],
                                start=(dc == 0),
                                stop=(dc == DC - 1),
                            )
                        for dc in range(DC):
                            nc.tensor.matmul(
                                psv[:, 512:768],
                                xkT[:, dc, jt * P : (jt + 1) * P],
                                wv_sb[:, dc, 512:768],
                                start=(dc == 0),
                                stop=(dc == DC - 1),
                            )
                        nc.vector.tensor_copy(
                            vaug[:, jt, :, 0:DH],
                            psv.rearrange("p (h d) -> p h d", h=H),
                        )
                        nc.vector.tensor_copy(vaug[:, jt, :, DH], ones_f)

                    # meanV over ALL keys -> mvT_sb [128, 6], scaled by 1/N
                    ps_mv = psmv.tile([1, D], f32, tag="ps_mv")
                    for jt in range(NJT_FULL):
                        nc.tensor.matmul(
                            ps_mv[:, 0:512],
                            ones_r,
                            vaug[:, jt, 0:8, 0:DH],
                            start=(jt == 0),
                            stop=(jt == NJT_FULL - 1),
                        )
                    for jt in range(NJT_FULL):
                        nc.tensor.matmul(
                            ps_mv[:, 512:768],
                            ones_r,
                            vaug[:, jt, 8:12, 0:DH],
                            start=(jt == 0),
                            stop=(jt == NJT_FULL - 1),
                        )
                    nc.vector.tensor_scalar_mul(mv_row, in0=ps_mv, scalar1=1.0 / N)
                    ps_mvt = psmv.tile([P, DC], f32, tag="ps_mvt")
                    for c in range(DC):
                        nc.tensor.transpose(
                            ps_mvt[:, c : c + 1],
                            mv_row[0:1, c * P : (c + 1) * P],
                            id1,
                        )
                    nc.vector.tensor_copy(mvT_sb, ps_mvt)

                # K projection (active key tiles only)
                with tc.tile_pool(name="wst3", bufs=2) as wst3, \
                     tc.tile_pool(name="psp3", bufs=3, space="PSUM") as psp3:
                    nch_sizes = []
                    off = 0
                    while off < nk:
                        sz = min(512, nk - off)
                        if nk - (off + sz) == 128:  # avoid a 128-wide tail
                            sz = 384
                        nch_sizes.append((off, sz))
                        off += sz
                    for hdt in range(DC):
                        wk_t = wst3.tile([P, DC, P], f32r, tag="wstream3")
                        nc.sync.dma_start(
                            out=wk_t,
                            in_=wk_d.rearrange("(c p) e -> p c e", p=P)[
                                :, :, hdt * P : (hdt + 1) * P
                            ].bitcast(f32r),
                        )
                        for off, sz in nch_sizes:
                            ps = psp3.tile([P, 512], f32, tag="psproj3")
                            for dc in range(DC):
                                nc.tensor.matmul(
                                    ps[:, 0:sz],
                                    wk_t[:, dc, :],
                                    xkT[:, dc, off : off + sz],
                                    start=(dc == 0),
                                    stop=(dc == DC - 1),
                                )
                            nc.vector.tensor_copy(
                                kT[:, hdt, off : off + sz], ps[:, 0:sz]
                            )

            # ---------------- phase 4: attention ----------------
            with tc.tile_pool(name="psS", bufs=2, space="PSUM") as psS_pool, \
                 tc.tile_pool(name="psO", bufs=2, space="PSUM") as psO_pool, \
                 tc.tile_pool(name="pts", bufs=3) as pts, \
                 tc.tile_pool(name="nrm", bufs=2) as nrm:
                for h in range(H):
                    hdt, hh = h // 2, h % 2
                    pbase = DH * hh
                    kT_h = kT[pbase : pbase + DH, hdt, :]
                    qT_h = qT[pbase : pbase + DH, hdt, :]
                    psO = psO_pool.tile([DH + 1, NQ], f32, tag="psO")
                    for jt in range(njt_act):
                        psS = psS_pool.tile([P, NQ], f32, tag="psS")
                        for q2 in range(NQ // 512):
                            nc.tensor.matmul(
                                psS[:, q2 * 512 : (q2 + 1) * 512],
                                kT_h[:, jt * P : (jt + 1) * P],
                                qT_h[:, q2 * 512 : (q2 + 1) * 512],
                                start=True,
                                stop=True,
                            )
                        pT = pts.tile([P, NQ], f32r, tag="pT")
                        nc.scalar.activation(
                            pT,
                            psS,
                            mybir.ActivationFunctionType.Exp,
                            bias=cmneg[:, jt : jt + 1],
                            scale=SCALE,
                        )
                        for q2 in range(NQ // 512):
                            nc.tensor.matmul(
                                psO[:, q2 * 512 : (q2 + 1) * 512],
                                vaug[:, jt, h, :],
                                pT[:, q2 * 512 : (q2 + 1) * 512],
                                start=(jt == 0),
                                stop=(jt == njt_act - 1),
                            )
                    # normalize: attnT_h = psO[0:64] * (1/s) with s = psO[64]
                    r_sb = nrm.tile([1, NQ], f32, tag="r_sb")
                    nc.vector.reciprocal(r_sb, psO[DH : DH + 1, :])
                    rb_sb = nrm.tile([DH, NQ], f32, tag="rb_sb")
                    nc.gpsimd.partition_broadcast(rb_sb, r_sb, channels=DH)
                    nc.vector.tensor_mul(
                        attnT[pbase : pbase + DH, hdt, :],
                        psO[0:DH, :],
                        rb_sb,
                    )

            # -------- phase 5: output projection + masked-query fill --------
            with tc.tile_pool(name="wo_pool", bufs=1) as wo_pool, \
                 tc.tile_pool(name="fin", bufs=3) as fin, \
                 tc.tile_pool(name="psF", bufs=2, space="PSUM") as psF_pool, \
                 tc.tile_pool(name="psU", bufs=1, space="PSUM") as psU_pool:
                wo_sb = wo_pool.tile([P, DC, D], f32r)
                nc.sync.dma_start(
                    out=wo_sb, in_=wo_d.rearrange("(c p) e -> p c e", p=P).bitcast(f32r)
                )
                # uniform_row = meanV @ Wo  [1, 768]
                ps_u1 = psU_pool.tile([1, D], f32, tag="ps_u1")
                for c in range(DC):
                    nc.tensor.matmul(
                        ps_u1[:, 0:512],
                        mvT_sb[:, c : c + 1],
                        wo_sb[:, c, 0:512],
                        start=(c == 0),
                        stop=(c == DC - 1),
                    )
                for c in range(DC):
                    nc.tensor.matmul(
                        ps_u1[:, 512:768],
                        mvT_sb[:, c : c + 1],
                        wo_sb[:, c, 512:768],
                        start=(c == 0),
                        stop=(c == DC - 1),
                    )
                urow_sb = fin.tile([1, D], f32r, tag="urow")
                nc.vector.tensor_copy(urow_sb, ps_u1)

                for it in range(NIT):
                    psF = psF_pool.tile([P, D], f32, tag="psF")
                    for c in range(DC):
                        nc.tensor.matmul(
                            psF[:, 0:512],
                            attnT[:, c, it * P : (it + 1) * P],
                            wo_sb[:, c, 0:512],
                            start=(c == 0),
                            stop=(c == DC - 1),
                        )
                    for c in range(DC):
                        nc.tensor.matmul(
                            psF[:, 512:768],
                            attnT[:, c, it * P : (it + 1) * P],
                            wo_sb[:, c, 512:768],
                            start=(c == 0),
                            stop=(c == DC - 1),
                        )
                    # uniform filler for masked queries: (1-rm01) (x) urow
                    psu = psU_pool.tile([P, D], f32, tag="psu")
                    nc.tensor.matmul(
                        psu[:, 0:512],
                        rmneg_row[0:1, it * P : (it + 1) * P],
                        urow_sb[0:1, 0:512],
                        start=True,
                        stop=True,
                    )
                    nc.tensor.matmul(
                        psu[:, 512:768],
                        rmneg_row[0:1, it * P : (it + 1) * P],
                        urow_sb[0:1, 512:768],
                        start=True,
                        stop=True,
                    )
                    sel_sb = fin.tile([P, D], f32, tag="sel")
                    nc.vector.tensor_scalar_mul(
                        sel_sb, in0=psF, scalar1=rm01[:, it : it + 1]
                    )
                    out_sb = fin.tile([P, D], f32, tag="outsb")
                    nc.vector.tensor_add(out_sb, sel_sb, psu)
                    nc.sync.dma_start(
                        out=out_d.ap()[it * P : (it + 1) * P, :], in_=out_sb
                    )

    nc.compile()
    _BUILD_CACHE[njt_act] = nc
    return nc


def _marshal(x, x_mask, Wq, Wk, Wv, Wo):
    """Build per-core input maps. Returns (in_maps, njt_act)."""
    x = np.asarray(x, dtype=np.float32)
    x_mask = np.asarray(x_mask).astype(bool)
    Wq = np.ascontiguousarray(np.asarray(Wq, dtype=np.float32))
    Wk = np.ascontiguousarray(np.asarray(Wk, dtype=np.float32))
    Wv = np.ascontiguousarray(np.asarray(Wv, dtype=np.float32))
    Wo = np.ascontiguousarray(np.asarray(Wo, dtype=np.float32))

    if SORT_KEYS:
        # per-batch stable sort: unmasked keys first
        orders = [np.argsort(~x_mask[b], kind="stable") for b in range(B)]
        counts = [int(x_mask[b].sum()) for b in range(B)]
        njt_act = max(1, -(-max(counts) // P))  # ceil(max unmasked / 128)
    else:
        orders = [np.arange(N) for _ in range(B)]
        njt_act = NJT_FULL

    in_maps = []
    for c in range(8):
        b, qh = c // 2, c % 2
        order = orders[b]
        xk = x[b][order]                       # [N, D] keys (sorted)
        mk = x_mask[b][order]                  # [N] key mask (sorted)
        xq = x[b, qh * NQ : (qh + 1) * NQ]     # [NQ, D] queries natural
        mq = x_mask[b, qh * NQ : (qh + 1) * NQ]

        cm = np.where(mk, 0.0, MASK_NEG).astype(np.float32)      # [N]
        cmnegT = np.ascontiguousarray(cm.reshape(NJT_FULL, P).T)  # [128, 16]
        rm = mq.astype(np.float32)                                # [NQ]
        rm01T = np.ascontiguousarray(rm.reshape(NIT, P).T)        # [128, 8]
        rmneg_row = np.ascontiguousarray((1.0 - rm).reshape(1, NQ))

        in_maps.append({
            "xkT": np.ascontiguousarray(xk.T),   # [768, 2048]
            "xqT": np.ascontiguousarray(xq.T),   # [768, 1024]
            "Wq": Wq, "Wk": Wk, "Wv": Wv, "Wo": Wo,
            "cmnegT": cmnegT,
            "rm01T": rm01T,
            "rmneg_row": rmneg_row,
        })
    return in_maps, njt_act


def run(x, x_mask, Wq, Wk, Wv, Wo, trace=False):
    """Run on 8 cores; returns (full_output, BassKernelResults)."""
    in_maps, njt_act = _marshal(x, x_mask, Wq, Wk, Wv, Wo)
    nc = build(njt_act)
    res = run_bass_kernel_spmd(nc, in_maps, core_ids=list(range(8)), trace=trace)
    out = np.empty((B, N, D), dtype=np.float32)
    for c in range(8):
        b, qh = c // 2, c % 2
        out[b, qh * NQ : (qh + 1) * NQ] = res.results[c]["out"]
    return out, res


def kernel(**inputs) -> np.ndarray:
    out, _ = run(
        inputs["x"], inputs["x_mask"],
        inputs["Wq"], inputs["Wk"], inputs["Wv"], inputs["Wo"],
        trace=False,
    )
    return out
